# revision 20
# baseline (speedup 1.0000x reference)
"""Multi-head attention Bass kernel for Trainium2, SPMD over 8 NeuronCores.

Problem: B=4, S=2048, D=1024, 16 heads x 64. Sharding: core = (batch b, head-group hg)
with b in 0..3, hg in 0..1 -> each core computes 8 heads of one batch.

v2 pipeline (all matmuls bf16 operands, fp32 PSUM accumulation):
  - Scores computed transposed (S^T[k, q]) one 128-key chunk at a time; the two
    heads of a pair run CONCURRENTLY in the PE array as row-tiles (K=64 each,
    auto tile_position (0,0)/(64,0) from base partitions).
  - Scores PSUM is a single persistent [128, 2(parity), 2(head), 512] tile
    (4 banks). Parity ping-pong: PE writes parity p^1 while ScalarE exps
    parity p with one N=1024 activation per kc (both heads at once).
  - Software pipeline: per kc group emit scores(g), exp(g), then AV(g-1)
    (one group delayed) so the PE never head-of-line blocks on exp(g), and
    ScalarE runs at full duty.
  - AV: lhsT=[V|1] stationary -> O^T[d(+denom), q] accumulated over kc
    (ones column makes row 64 the softmax denominator).
  - QKV projections and output finalize interleave as filler closures in the
    filler budget of each group.
"""
import os
import numpy as np
import ml_dtypes
from contextlib import ExitStack

import concourse.tile as tile
import concourse.mybir as mybir
from concourse import bacc
from concourse.bass_utils import run_bass_kernel_spmd

P = 128
DH = 64
BF = mybir.dt.bfloat16
F32 = mybir.dt.float32


def build_attention(S=2048, D=1024, HPC=8, loop_n=1, ablate=(), pbufs=4, pops=1):
    """Build the per-core SPMD program. HPC = heads per core (even).

    loop_n > 1 wraps the whole body in a hardware loop (for timing)."""
    DC = D // P        # D chunks of 128
    KC = S // P        # k chunks of 128
    NQ = S // 512      # q blocks of 512
    HP = HPC // 2      # head pairs
    CW = HPC * DH      # core output width
    SCALE = 1.0 / float(np.sqrt(DH))

    nc = bacc.Bacc("TRN2")
    xq = nc.dram_tensor("xq", [DC, P, S], BF, kind="ExternalInput")
    xk = nc.dram_tensor("xk", [DC, P, S], BF, kind="ExternalInput")
    xv = nc.dram_tensor("xv", [DC, P, S], BF, kind="ExternalInput")
    wq = nc.dram_tensor("wq", [DC, P, CW], BF, kind="ExternalInput")
    wk = nc.dram_tensor("wk", [DC, P, CW], BF, kind="ExternalInput")
    wv = nc.dram_tensor("wv", [DC, P, CW], BF, kind="ExternalInput")
    out = nc.dram_tensor("out", [HPC, DH, S], F32, kind="ExternalOutput")

    with tile.TileContext(nc) as tc, ExitStack() as ctx:
        xpool = ctx.enter_context(tc.tile_pool(name="x", bufs=1))
        wpool = ctx.enter_context(tc.tile_pool(name="w", bufs=1))
        vpool = ctx.enter_context(tc.tile_pool(name="v", bufs=1))
        qkpool = ctx.enter_context(tc.tile_pool(name="qk", bufs=2))
        ppool = ctx.enter_context(tc.tile_pool(name="p", bufs=pbufs))
        ostag = ctx.enter_context(tc.tile_pool(name="ost", bufs=4))
        outp = ctx.enter_context(tc.tile_pool(name="outp", bufs=4))
        rpool = ctx.enter_context(tc.tile_pool(name="r", bufs=4))
        ps_ss = ctx.enter_context(tc.tile_pool(name="ps_ss", bufs=2, space="PSUM"))
        ps_o = ctx.enter_context(tc.tile_pool(name="ps_o", bufs=1, space="PSUM"))
        ps_m = ctx.enter_context(tc.tile_pool(name="ps_m", bufs=2, space="PSUM"))

        xs, ws = {}, {}
        vt = None

        def emit_loads():
            for name, dram in [("q", wq), ("k", wk), ("v", wv)]:
                t = wpool.tile([P, DC, CW], BF, tag="w" + name, name="w" + name)
                for dc in range(DC):
                    nc.sync.dma_start(t[:, dc, :], dram[dc])
                ws[name] = t
            for name, dram in [("q", xq), ("k", xk), ("v", xv)]:
                t = xpool.tile([P, DC, S], BF, tag="x" + name, name="x" + name)
                for dc in range(DC):
                    nc.sync.dma_start(t[:, dc, :], dram[dc])
                xs[name] = t
            # V for all heads: [p(k in chunk), kc, ch, 0:DH] = V, [..., DH] = 1.0
            nonlocal vt
            vt = vpool.tile([P, KC, HPC, DH + 1], BF, tag="V", name="vt")
            nc.any.memset(vt[:, :, :, DH : DH + 1], 1.0)

        def proj_v_kc(kc):
            pv = ps_m.tile([P, 512], F32, tag="proj", name="pv")[:, :CW]
            for dc in range(DC):
                nc.tensor.matmul(
                    pv,
                    xs["v"][:, dc, kc * P : (kc + 1) * P],
                    ws["v"][:, dc, :],
                    start=(dc == 0),
                    stop=(dc == DC - 1),
                )
            nc.vector.tensor_copy(
                vt[:, kc, :, 0:DH],
                pv.rearrange("p (h d) -> p h d", d=DH),
            )

        def proj_qk_chunk(t, which, hp, qb):
            pp = ps_m.tile([P, 512], F32, tag="proj")
            for dc in range(DC):
                nc.tensor.matmul(
                    pp[:],
                    ws[which][:, dc, hp * P : (hp + 1) * P],
                    xs[which][:, dc, qb * 512 : (qb + 1) * 512],
                    start=(dc == 0),
                    stop=(dc == DC - 1),
                )
            nc.vector.tensor_copy(t[:, qb * 512 : (qb + 1) * 512], pp[:])

        def new_qk(which):
            return qkpool.tile([P, S], BF, tag=which, name=which + "t")

        def proj_qk_fillers(t, which, hp, qb, key):
            """Projection of one 512-wide chunk of q^T or k^T for head pair hp
            as keyed filler closures (2 accumulating matmuls each + final
            evacuate). `key` = (hp, qb, kc) of the first scores matmul that
            READS the chunk: for q that is (hp, qb, 0); for k — whose chunk
            covers KEYS [512qb, 512qb+512), read by every q block at
            kc = 4qb..4qb+3 — it is (hp, 0, 4qb)."""
            state = {}

            def mk(dc0):
                def f():
                    if dc0 == 0:
                        state["pp"] = ps_m.tile([P, 512], F32, tag="proj",
                                                name="pp")
                    pp = state["pp"]
                    for dc in (dc0, dc0 + 1):
                        nc.tensor.matmul(
                            pp[:],
                            ws[which][:, dc, hp * P : (hp + 1) * P],
                            xs[which][:, dc, qb * 512 : (qb + 1) * 512],
                            start=(dc == 0),
                            stop=(dc == DC - 1),
                        )
                    if dc0 == DC - 2:
                        nc.vector.tensor_copy(
                            t[:, qb * 512 : (qb + 1) * 512], pp[:])
                return f

            return [(key, mk(d)) for d in range(0, DC, 2)]

        def finalize_fillers(osbs, hp, qb):
            """Reciprocal + broadcast + normalize + store for one finished q
            block, as filler closures."""
            fillers = []
            if "nofin" in ablate:
                return []
            for h in (0, 1):
                ch = hp * 2 + h
                osb = osbs[h]
                state = {}

                def rec(osb=osb, state=state):
                    rsb = rpool.tile([1, 512], F32, tag="rc", name="rsb")
                    nc.vector.reciprocal(rsb[:], osb[DH : DH + 1, :])
                    rbc = rpool.tile([DH, 512], F32, tag="rbc", name="rbc")
                    nc.gpsimd.partition_broadcast(rbc[:], rsb[0:1, :])
                    state["rbc"] = rbc

                def norm(ch=ch, qb=qb, osb=osb, state=state):
                    ot = outp.tile([DH, 512], F32, tag="ot", name="ot")
                    nc.vector.tensor_tensor(
                        ot[:], osb[0:DH, :], state["rbc"][:], mybir.AluOpType.mult)
                    nc.sync.dma_start(
                        out[ch, :, qb * 512 : (qb + 1) * 512], ot[:])

                fillers += [rec, norm]
            return fillers

        def emit_body():
            emit_loads()
            qt = new_qk("q")
            kt = new_qk("k")
            # prologue: only the first q block of head pair 0
            # prologue: q block 0 of head pair 0, and ALL of k^T (scores for
            # every q block walk the full key range)
            proj_qk_chunk(qt, "q", 0, 0)
            for qb in range(NQ):
                proj_qk_chunk(kt, "k", 0, qb)
            if "vpre" in ablate:
                for kc in range(KC):
                    proj_v_kc(kc)
            # proj_q stays sorted by key (hp, qb, kc)
            proj_q = []
            for qb in range(1, NQ):
                proj_q += proj_qk_fillers(qt, "q", 0, qb, (0, qb, 0))
            fin_q = []
            state = {"pend": None}

            def drain_proj(key):
                while proj_q and proj_q[0][0] <= key:
                    proj_q.pop(0)[1]()

            for hp in range(HP):
                if hp + 1 < HP and "noproj" not in ablate:
                    qt_next = new_qk("q")
                    kt_next = new_qk("k")
                    # sorted by key: (hp+1,0,0) q0,k0 then k1..k3 at the kc
                    # that first reads them, then q1..q3 at their qb starts
                    proj_q += proj_qk_fillers(qt_next, "q", hp + 1, 0,
                                              (hp + 1, 0, 0))
                    proj_q += proj_qk_fillers(kt_next, "k", hp + 1, 0,
                                              (hp + 1, 0, 0))
                    for qb in range(1, NQ):
                        proj_q += proj_qk_fillers(kt_next, "k", hp + 1, qb,
                                                  (hp + 1, 0, 4 * qb))
                    for qb in range(1, NQ):
                        proj_q += proj_qk_fillers(qt_next, "q", hp + 1, qb,
                                                  (hp + 1, qb, 0))
                elif hp + 1 < HP:
                    qt_next, kt_next = qt, kt
                for qb in range(NQ):
                    o_ps = [ps_o.tile([DH + 1, 512], F32, tag=f"O{h}",
                                      name=f"O{h}") for h in (0, 1)]
                    for kc in range(KC):
                        # anything this group's scores read must be emitted now
                        drain_proj((hp, qb, kc))
                        # double-buffered scores PSUM (pool cycles 2 tiles =
                        # 4 banks): PE writes tile g+1 while ScalarE exps g
                        ss = ps_ss.tile([P, 2, 512], F32, tag="SS", name="SS")
                        # scores: both heads concurrent as PE row-tiles
                        for h in (0, 1):
                            nc.tensor.matmul(
                                ss[:, h, :],
                                kt[h * DH : (h + 1) * DH, kc * P : (kc + 1) * P],
                                qt[h * DH : (h + 1) * DH, qb * 512 : (qb + 1) * 512],
                                start=True,
                                stop=True,
                            )
                        # V projection just-in-time during the first pass
                        if hp == 0 and qb == 0 and "vpre" not in ablate:
                            proj_v_kc(kc)
                        # exp of both heads in one N=1024 activation
                        pt = ppool.tile([P, 2, 512], BF, tag="pt", name="pt")
                        if "noexp" not in ablate:
                            nc.scalar.activation(
                                pt[:], ss[:],
                                mybir.ActivationFunctionType.Exp, scale=SCALE)
                        else:
                            nc.vector.tensor_copy(pt[:, 0, :16], ss[:, 0, :16])
                        # flush the previous group's AV while exp(g) runs
                        if state["pend"] is not None:
                            state["pend"]()

                        def mk_av(pt=pt, o_ps=o_ps, kc=kc, hp=hp):
                            def f():
                                if "noav" in ablate:
                                    return
                                for h in (0, 1):
                                    ch = hp * 2 + h
                                    nc.tensor.matmul(
                                        o_ps[h][:],
                                        vt[:, kc, ch, :],
                                        pt[:, h, :],
                                        start=(kc == 0),
                                        stop=(kc == KC - 1),
                                    )
                            return f

                        state["pend"] = mk_av()
                        # interleave deferred work in the ScalarE slack:
                        # proj fillers are PE matmuls (pace them carefully),
                        # finalize fillers are DVE/GPSIMD-only (nearly free)
                        if not (hp == 0 and qb == 0):
                            budget = pops
                            while budget and proj_q:
                                proj_q.pop(0)[1]()
                                budget -= 1
                        budget = 2
                        while budget and fin_q:
                            fin_q.pop(0)()
                            budget -= 1
                    # chain: AV(kc=15) then evacuate O PSUM, emitted at the
                    # start of the next group so the PE keeps its lead
                    prev = state["pend"]
                    osbs = []
                    for h in (0, 1):
                        osbs.append(ostag.tile([DH + 1, 512], F32, tag="osb",
                                               name="osb"))

                    def mk_chain(prev=prev, osbs=osbs, o_ps=o_ps):
                        def f():
                            prev()
                            for h in (0, 1):
                                nc.vector.tensor_copy(osbs[h][:], o_ps[h][:])
                        return f

                    state["pend"] = mk_chain()
                    fin_q += finalize_fillers(osbs, hp, qb)
                if hp + 1 < HP:
                    qt, kt = qt_next, kt_next
            state["pend"]()
            for f in fin_q:
                f()

        if loop_n > 1:
            with tc.For_i(0, loop_n, 1):
                emit_body()
        else:
            emit_body()

    nc.compile()
    return nc


_NC_CACHE = {}


def _get_nc(S, D, HPC):
    key = (S, D, HPC)
    if key not in _NC_CACHE:
        ablate = tuple(filter(None, os.environ.get("KABLATE", "").split(",")))
        _NC_CACHE[key] = build_attention(S, D, HPC, ablate=ablate)
    return _NC_CACHE[key]


def _prep_core_inputs(q_seq, k_seq, v_seq, WQ, WK, WV, b, hg, HPC, D):
    """Host-side shard prep for core (batch b, head group hg)."""
    DC = D // P
    CW = HPC * DH
    bf16 = ml_dtypes.bfloat16

    def xt(x):  # [S, D] -> [DC, P, S] (D-major transpose)
        return np.ascontiguousarray(x.T.reshape(DC, P, -1)).astype(bf16)

    def wslice(w):  # [D, out] -> [DC, P, CW]
        return np.ascontiguousarray(
            w[:, hg * CW : (hg + 1) * CW].reshape(DC, P, CW)
        ).astype(bf16)

    return {
        "xq": xt(q_seq[b]),
        "xk": xt(k_seq[b]),
        "xv": xt(v_seq[b]),
        "wq": wslice(WQ),
        "wk": wslice(WK),
        "wv": wslice(WV),
    }


def kernel(q_seq, k_seq, v_seq, WQ, WK, WV, _trace=False):
    q_seq = np.asarray(q_seq, dtype=np.float32)
    k_seq = np.asarray(k_seq, dtype=np.float32)
    v_seq = np.asarray(v_seq, dtype=np.float32)
    WQ = np.asarray(WQ, dtype=np.float32)
    WK = np.asarray(WK, dtype=np.float32)
    WV = np.asarray(WV, dtype=np.float32)

    B, S, D = q_seq.shape
    NB_HEAD = WQ.shape[1] // DH
    n_cores = 8
    groups_per_batch = n_cores // B          # 2 head groups
    HPC = NB_HEAD // groups_per_batch        # 8 heads per core
    CW = HPC * DH

    nc = _get_nc(S, D, HPC)

    in_maps = []
    for core in range(n_cores):
        b, hg = core // groups_per_batch, core % groups_per_batch
        in_maps.append(_prep_core_inputs(q_seq, k_seq, v_seq, WQ, WK, WV, b, hg, HPC, D))

    res = run_bass_kernel_spmd(
        nc, in_maps, core_ids=list(range(n_cores)), trace=_trace,
        **({"trace_cores": [0], } if _trace else {}),
    )
    if _trace:
        print(f"HW exec time: {res.exec_time_ns} ns")
        if res.instructions_and_trace:
            print("trace:", res.instructions_and_trace[1])

    out = np.empty((B, S, NB_HEAD * DH), dtype=np.float32)
    for core in range(n_cores):
        b, hg = core // groups_per_batch, core % groups_per_batch
        # device output is O^T per head: [HPC, DH, S] -> [S, HPC*DH]
        ot = res.results[core]["out"]
        out[b, :, hg * CW : (hg + 1) * CW] = (
            ot.transpose(2, 0, 1).reshape(S, CW)
        )
    return out


# revision 26
# speedup vs baseline: 1.1942x; 1.1942x over previous
"""Multi-head attention Bass kernel for Trainium2, SPMD over 8 NeuronCores.

Problem: B=4, S=2048, D=1024, 16 heads x 64. Sharding: core = (batch b, head-group hg)
with b in 0..3, hg in 0..1 -> each core computes 8 heads of one batch.

v2 pipeline (all matmuls bf16 operands, fp32 PSUM accumulation):
  - Scores computed transposed (S^T[k, q]) one 128-key chunk at a time; the two
    heads of a pair run CONCURRENTLY in the PE array as row-tiles (K=64 each,
    auto tile_position (0,0)/(64,0) from base partitions).
  - Scores PSUM is a single persistent [128, 2(parity), 2(head), 512] tile
    (4 banks). Parity ping-pong: PE writes parity p^1 while ScalarE exps
    parity p with one N=1024 activation per kc (both heads at once).
  - Software pipeline: per kc group emit scores(g), exp(g), then AV(g-1)
    (one group delayed) so the PE never head-of-line blocks on exp(g), and
    ScalarE runs at full duty.
  - AV: lhsT=[V|1] stationary -> O^T[d(+denom), q] accumulated over kc
    (ones column makes row 64 the softmax denominator).
  - QKV projections and output finalize interleave as filler closures in the
    filler budget of each group.
"""
import os
import numpy as np
import ml_dtypes
from contextlib import ExitStack

import concourse.tile as tile
import concourse.mybir as mybir
from concourse import bacc
from concourse.bass_utils import run_bass_kernel_spmd

P = 128
DH = 64
BF = mybir.dt.bfloat16
F32 = mybir.dt.float32


def build_attention(S=2048, D=1024, HPC=8, loop_n=1, ablate=(), pbufs=4, pops=2):
    """Build the per-core SPMD program. HPC = heads per core (even).

    loop_n > 1 wraps the whole body in a hardware loop (for timing)."""
    DC = D // P        # D chunks of 128
    KC = S // P        # k chunks of 128
    NQ = S // 512      # q blocks of 512
    HP = HPC // 2      # head pairs
    CW = HPC * DH      # core output width
    SCALE = 1.0 / float(np.sqrt(DH))

    nc = bacc.Bacc("TRN2")
    xq = nc.dram_tensor("xq", [DC, P, S], BF, kind="ExternalInput")
    xk = nc.dram_tensor("xk", [DC, P, S], BF, kind="ExternalInput")
    xv = nc.dram_tensor("xv", [DC, P, S], BF, kind="ExternalInput")
    wq = nc.dram_tensor("wq", [DC, P, CW], BF, kind="ExternalInput")
    wk = nc.dram_tensor("wk", [DC, P, CW], BF, kind="ExternalInput")
    wv = nc.dram_tensor("wv", [DC, P, CW], BF, kind="ExternalInput")
    out = nc.dram_tensor("out", [HPC, DH, S], F32, kind="ExternalOutput")

    with tile.TileContext(nc) as tc, ExitStack() as ctx:
        xpool = ctx.enter_context(tc.tile_pool(name="x", bufs=1))
        wpool = ctx.enter_context(tc.tile_pool(name="w", bufs=1))
        vpool = ctx.enter_context(tc.tile_pool(name="v", bufs=1))
        qkpool = ctx.enter_context(tc.tile_pool(name="qk", bufs=2))
        ppool = ctx.enter_context(tc.tile_pool(name="p", bufs=pbufs))
        ostag = ctx.enter_context(tc.tile_pool(name="ost", bufs=4))
        outp = ctx.enter_context(tc.tile_pool(name="outp", bufs=4))
        rpool = ctx.enter_context(tc.tile_pool(name="r", bufs=4))
        ps_ss = ctx.enter_context(tc.tile_pool(name="ps_ss", bufs=2, space="PSUM"))
        ps_o = ctx.enter_context(tc.tile_pool(name="ps_o", bufs=1, space="PSUM"))
        ps_m = ctx.enter_context(tc.tile_pool(name="ps_m", bufs=2, space="PSUM"))

        xs, ws = {}, {}
        vt = None

        def emit_loads():
            for name, dram in [("q", wq), ("k", wk), ("v", wv)]:
                t = wpool.tile([P, DC, CW], BF, tag="w" + name, name="w" + name)
                for dc in range(DC):
                    nc.sync.dma_start(t[:, dc, :], dram[dc])
                ws[name] = t
            for name, dram in [("q", xq), ("k", xk), ("v", xv)]:
                t = xpool.tile([P, DC, S], BF, tag="x" + name, name="x" + name)
                for dc in range(DC):
                    nc.sync.dma_start(t[:, dc, :], dram[dc])
                xs[name] = t
            # V for all heads: [p(k in chunk), kc, ch, 0:DH] = V, [..., DH] = 1.0
            nonlocal vt
            vt = vpool.tile([P, KC, HPC, DH + 1], BF, tag="V", name="vt")
            nc.any.memset(vt[:, :, :, DH : DH + 1], 1.0)

        def proj_v_kc(kc):
            pv = ps_m.tile([P, 512], F32, tag="proj", name="pv")[:, :CW]
            for dc in range(DC):
                nc.tensor.matmul(
                    pv,
                    xs["v"][:, dc, kc * P : (kc + 1) * P],
                    ws["v"][:, dc, :],
                    start=(dc == 0),
                    stop=(dc == DC - 1),
                )
            nc.vector.tensor_copy(
                vt[:, kc, :, 0:DH],
                pv.rearrange("p (h d) -> p h d", d=DH),
            )

        def proj_qk_chunk(t, which, hp, qb):
            pp = ps_m.tile([P, 512], F32, tag="proj")
            for dc in range(DC):
                nc.tensor.matmul(
                    pp[:],
                    ws[which][:, dc, hp * P : (hp + 1) * P],
                    xs[which][:, dc, qb * 512 : (qb + 1) * 512],
                    start=(dc == 0),
                    stop=(dc == DC - 1),
                )
            nc.vector.tensor_copy(t[:, qb * 512 : (qb + 1) * 512], pp[:])

        def new_qk(which):
            return qkpool.tile([P, S], BF, tag=which, name=which + "t")

        def proj_qk_fillers(t, which, hp, qb, key):
            """Projection of one 512-wide chunk of q^T or k^T for head pair hp
            as keyed filler closures (2 accumulating matmuls each + final
            evacuate). `key` = (hp, qb, kc) of the first scores matmul that
            READS the chunk: for q that is (hp, qb, 0); for k — whose chunk
            covers KEYS [512qb, 512qb+512), read by every q block at
            kc = 4qb..4qb+3 — it is (hp, 0, 4qb)."""
            state = {}

            def mk(dc0):
                def f():
                    if dc0 == 0:
                        state["pp"] = ps_m.tile([P, 512], F32, tag="proj",
                                                name="pp")
                    pp = state["pp"]
                    for dc in (dc0, dc0 + 1):
                        nc.tensor.matmul(
                            pp[:],
                            ws[which][:, dc, hp * P : (hp + 1) * P],
                            xs[which][:, dc, qb * 512 : (qb + 1) * 512],
                            start=(dc == 0),
                            stop=(dc == DC - 1),
                        )
                    if dc0 == DC - 2:
                        nc.vector.tensor_copy(
                            t[:, qb * 512 : (qb + 1) * 512], pp[:])
                return f

            return [(key, mk(d)) for d in range(0, DC, 2)]

        def finalize_fillers(osbs, hp, qb):
            """Reciprocal + broadcast + normalize + store for one finished q
            block, as filler closures."""
            fillers = []
            if "nofin" in ablate:
                return []
            for h in (0, 1):
                ch = hp * 2 + h
                osb = osbs[h]
                state = {}

                def rec(osb=osb, state=state):
                    rsb = rpool.tile([1, 512], F32, tag="rc", name="rsb")
                    nc.vector.reciprocal(rsb[:], osb[DH : DH + 1, :])
                    rbc = rpool.tile([DH, 512], F32, tag="rbc", name="rbc")
                    nc.gpsimd.partition_broadcast(rbc[:], rsb[0:1, :])
                    state["rbc"] = rbc

                def norm(ch=ch, qb=qb, osb=osb, state=state):
                    ot = outp.tile([DH, 512], F32, tag="ot", name="ot")
                    nc.vector.tensor_tensor(
                        ot[:], osb[0:DH, :], state["rbc"][:], mybir.AluOpType.mult)
                    nc.sync.dma_start(
                        out[ch, :, qb * 512 : (qb + 1) * 512], ot[:])

                fillers += [rec, norm]
            return fillers

        def emit_body():
            emit_loads()
            if "dmaonly" in ablate:
                return
            # chaincut/noexp: AV consumes a constant tile so the
            # scores->exp->AV dependency chain is cut (engine loads intact
            # for chaincut; noexp also drops the activation itself)
            pt_dummy = None
            if "chaincut" in ablate or "noexp" in ablate:
                pt_dummy = vpool.tile([P, 2, 512], BF, tag="ptd", name="ptd")
                nc.any.memset(pt_dummy[:], 0.5)
            qt = new_qk("q")
            kt = new_qk("k")
            # prologue: only the first q block of head pair 0
            # prologue: q block 0 of head pair 0, and ALL of k^T (scores for
            # every q block walk the full key range)
            proj_qk_chunk(qt, "q", 0, 0)
            for qb in range(NQ):
                proj_qk_chunk(kt, "k", 0, qb)
            if "vpre" in ablate:
                for kc in range(KC):
                    proj_v_kc(kc)
            # proj_q stays sorted by key (hp, qb, kc)
            proj_q = []
            for qb in range(1, NQ):
                proj_q += proj_qk_fillers(qt, "q", 0, qb, (0, qb, 0))
            fin_q = []
            state = {"pend": None}

            def drain_proj(key):
                while proj_q and proj_q[0][0] <= key:
                    proj_q.pop(0)[1]()

            for hp in range(HP):
                if hp + 1 < HP and "noproj" not in ablate:
                    qt_next = new_qk("q")
                    kt_next = new_qk("k")
                    # sorted by key: (hp+1,0,0) q0,k0 then k1..k3 at the kc
                    # that first reads them, then q1..q3 at their qb starts
                    proj_q += proj_qk_fillers(qt_next, "q", hp + 1, 0,
                                              (hp + 1, 0, 0))
                    proj_q += proj_qk_fillers(kt_next, "k", hp + 1, 0,
                                              (hp + 1, 0, 0))
                    for qb in range(1, NQ):
                        proj_q += proj_qk_fillers(kt_next, "k", hp + 1, qb,
                                                  (hp + 1, 0, 4 * qb))
                    for qb in range(1, NQ):
                        proj_q += proj_qk_fillers(qt_next, "q", hp + 1, qb,
                                                  (hp + 1, qb, 0))
                elif hp + 1 < HP:
                    qt_next, kt_next = qt, kt
                for qb in range(NQ):
                    o_ps = [ps_o.tile([DH + 1, 512], F32, tag=f"O{h}",
                                      name=f"O{h}") for h in (0, 1)]
                    if "noav" in ablate:
                        for h in (0, 1):
                            nc.any.memset(o_ps[h][:], 1.0)
                    for kcp in range(KC // 2):
                        kcs = (2 * kcp, 2 * kcp + 1)
                        # anything this beat's scores read must be emitted now
                        drain_proj((hp, qb, kcs[1]))
                        # scores burst: 4 MMs back-to-back (2 kc x 2 heads);
                        # the two ss tiles ARE the pool's double buffer, so
                        # the next beat's scores overlap this beat's exps
                        sss = []
                        for kc in kcs:
                            ss = ps_ss.tile([P, 2, 512], F32, tag="SS",
                                            name="SS")
                            for h in (0, 1):
                                nc.tensor.matmul(
                                    ss[:, h, :],
                                    kt[h * DH : (h + 1) * DH,
                                       kc * P : (kc + 1) * P],
                                    qt[h * DH : (h + 1) * DH,
                                       qb * 512 : (qb + 1) * 512],
                                    start=True,
                                    stop=True,
                                )
                            sss.append(ss)
                        # V projection just-in-time during the first pass
                        if hp == 0 and qb == 0 and "vpre" not in ablate:
                            proj_v_kc(kcs[0])
                            proj_v_kc(kcs[1])
                        # exp of both heads in one N=1024 activation per kc
                        pts = []
                        for kc, ss in zip(kcs, sss):
                            pt = None
                            if "noexp" not in ablate:
                                pt = ppool.tile([P, 2, 512], BF, tag="pt",
                                                name="pt")
                                nc.scalar.activation(
                                    pt[:], ss[:],
                                    mybir.ActivationFunctionType.Exp,
                                    scale=SCALE)
                            if pt_dummy is not None:
                                pt = pt_dummy
                            pts.append(pt)
                        # flush the previous beat's AV burst (4 MMs) while
                        # this beat's exps run
                        if state["pend"] is not None:
                            state["pend"]()

                        def mk_av(pts=pts, o_ps=o_ps, kcs=kcs, hp=hp):
                            def f():
                                if "noav" in ablate:
                                    return
                                for kc, pt in zip(kcs, pts):
                                    for h in (0, 1):
                                        ch = hp * 2 + h
                                        nc.tensor.matmul(
                                            o_ps[h][:],
                                            vt[:, kc, ch, :],
                                            pt[:, h, :],
                                            start=(kc == 0),
                                            stop=(kc == KC - 1),
                                        )
                            return f

                        state["pend"] = mk_av()
                        # interleave deferred work in the ScalarE slack:
                        # proj fillers are PE matmuls (pace them carefully),
                        # finalize fillers are DVE/GPSIMD-only (nearly free)
                        if not (hp == 0 and qb == 0):
                            budget = pops
                            while budget and proj_q:
                                proj_q.pop(0)[1]()
                                budget -= 1
                        budget = 2
                        while budget and fin_q:
                            fin_q.pop(0)()
                            budget -= 1
                    # chain: AV(kc=15) then evacuate O PSUM, emitted at the
                    # start of the next group so the PE keeps its lead
                    prev = state["pend"]
                    osbs = []
                    for h in (0, 1):
                        osbs.append(ostag.tile([DH + 1, 512], F32, tag="osb",
                                               name="osb"))

                    def mk_chain(prev=prev, osbs=osbs, o_ps=o_ps):
                        def f():
                            prev()
                            for h in (0, 1):
                                nc.vector.tensor_copy(osbs[h][:], o_ps[h][:])
                        return f

                    state["pend"] = mk_chain()
                    fin_q += finalize_fillers(osbs, hp, qb)
                if hp + 1 < HP:
                    qt, kt = qt_next, kt_next
            state["pend"]()
            for f in fin_q:
                f()

        if loop_n > 1:
            with tc.For_i(0, loop_n, 1):
                emit_body()
        else:
            emit_body()

    nc.compile()
    return nc


_NC_CACHE = {}


def _get_nc(S, D, HPC):
    key = (S, D, HPC)
    if key not in _NC_CACHE:
        ablate = tuple(filter(None, os.environ.get("KABLATE", "").split(",")))
        _NC_CACHE[key] = build_attention(S, D, HPC, ablate=ablate)
    return _NC_CACHE[key]


def _prep_core_inputs(q_seq, k_seq, v_seq, WQ, WK, WV, b, hg, HPC, D):
    """Host-side shard prep for core (batch b, head group hg)."""
    DC = D // P
    CW = HPC * DH
    bf16 = ml_dtypes.bfloat16

    def xt(x):  # [S, D] -> [DC, P, S] (D-major transpose)
        return np.ascontiguousarray(x.T.reshape(DC, P, -1)).astype(bf16)

    def wslice(w):  # [D, out] -> [DC, P, CW]
        return np.ascontiguousarray(
            w[:, hg * CW : (hg + 1) * CW].reshape(DC, P, CW)
        ).astype(bf16)

    return {
        "xq": xt(q_seq[b]),
        "xk": xt(k_seq[b]),
        "xv": xt(v_seq[b]),
        "wq": wslice(WQ),
        "wk": wslice(WK),
        "wv": wslice(WV),
    }


def kernel(q_seq, k_seq, v_seq, WQ, WK, WV, _trace=False):
    q_seq = np.asarray(q_seq, dtype=np.float32)
    k_seq = np.asarray(k_seq, dtype=np.float32)
    v_seq = np.asarray(v_seq, dtype=np.float32)
    WQ = np.asarray(WQ, dtype=np.float32)
    WK = np.asarray(WK, dtype=np.float32)
    WV = np.asarray(WV, dtype=np.float32)

    B, S, D = q_seq.shape
    NB_HEAD = WQ.shape[1] // DH
    n_cores = 8
    groups_per_batch = n_cores // B          # 2 head groups
    HPC = NB_HEAD // groups_per_batch        # 8 heads per core
    CW = HPC * DH

    nc = _get_nc(S, D, HPC)

    in_maps = []
    for core in range(n_cores):
        b, hg = core // groups_per_batch, core % groups_per_batch
        in_maps.append(_prep_core_inputs(q_seq, k_seq, v_seq, WQ, WK, WV, b, hg, HPC, D))

    res = run_bass_kernel_spmd(
        nc, in_maps, core_ids=list(range(n_cores)), trace=_trace,
        **({"trace_cores": [0], } if _trace else {}),
    )
    if _trace:
        print(f"HW exec time: {res.exec_time_ns} ns")
        if res.instructions_and_trace:
            print("trace:", res.instructions_and_trace[1])

    out = np.empty((B, S, NB_HEAD * DH), dtype=np.float32)
    for core in range(n_cores):
        b, hg = core // groups_per_batch, core % groups_per_batch
        # device output is O^T per head: [HPC, DH, S] -> [S, HPC*DH]
        ot = res.results[core]["out"]
        out[b, :, hg * CW : (hg + 1) * CW] = (
            ot.transpose(2, 0, 1).reshape(S, CW)
        )
    return out
